# revision 1
# baseline (speedup 1.0000x reference)
"""Trainium2 Bass kernel for the BDH-style sparse-attention network.

Reference computation (per batch b, all fp32):
  v = LN(wte[idx])                                   [T, D]
  repeat L times:
    x   = relu(v @ Dx)                               [T, N]   (Dx: [D, N] = decoder_x heads concat)
    a   = causal_linear_attention(v) (RoPE, no softmax, tril mask)
    y   = relu(LN(a) @ Dy) * x                       [T, N]
    v   = v + LN(y @ E)                              [T, D]   (E: [N, D] = encoder)
  logits = v @ readout                               [T, VOCAB]

Sharding over 8 NeuronCores: core c -> batch b = c//2, neuron half h = c%2.
Each core holds half the neuron dim (N/2 columns of Dx/Dy, N/2 rows of E) and
computes the full attention for its batch; the partial `y @ E` update is
summed with an AllReduce over core pairs [[0,1],[2,3],[4,5],[6,7]].

On-device layout: token-major tiles v [128t, D] plus a transposed copy
vT [128d, T] maintained via PE transposes, so every matmul has its
contraction dim on partitions without extra data movement.
"""

import numpy as np

import concourse.bass as bass
import concourse.bacc as bacc
import concourse.mybir as mybir
import concourse.tile as tile
from concourse.bass_utils import run_bass_kernel_spmd

FP = mybir.dt.float32
AX = mybir.AxisListType
ALU = mybir.AluOpType
ACTF = mybir.ActivationFunctionType
EPS = 1e-5


def default_cfg():
    return dict(
        T=2048, D=256, N=8192, H=4, VOCAB=256, L=6, B=4,
        TCHUNK=512,          # tokens per chunk == attention query block
        mm_dt="f32r",        # "f32r" | "f32" : dtype view fed to the PE
        w_dt="mm",           # "bf16" | "mm" : matmul dtype for the MLP path
        n_cores=8,
        reps=1,              # layer-stack repeats (for wall-clock timing deltas)
    )


def build_program(cfg):
    """Builds and compiles the per-core SPMD bass program."""
    T, D, VOCAB, L = cfg["T"], cfg["D"], cfg["VOCAB"], cfg["L"]
    NH = cfg["N"] // 2
    TC = cfg["TCHUNK"]
    TT = T // 128
    DT = D // 128
    VT = VOCAB // 128
    n_cores = cfg["n_cores"]
    assert D == 256 and TC % 128 == 0 and T % TC == 0 and T % 512 == 0

    MDT = mybir.dt.float32r if cfg["mm_dt"] == "f32r" else FP
    WDT = mybir.dt.bfloat16 if cfg.get("w_dt") == "bf16" else MDT

    nc = bacc.Bacc("TRN2", target_bir_lowering=False, debug=False,
                   num_devices=n_cores)

    idxf_d = nc.dram_tensor("idxf", [1, T], FP, kind="ExternalInput")
    wte_d = nc.dram_tensor("wte", [VT, 128, D], FP, kind="ExternalInput")
    dxh_d = nc.dram_tensor("dxh", [DT, 128, NH], WDT, kind="ExternalInput")
    dyh_d = nc.dram_tensor("dyh", [DT, 128, NH], WDT, kind="ExternalInput")
    eh_d = nc.dram_tensor("eh", [NH // 128, 128, D], WDT, kind="ExternalInput")
    ro_d = nc.dram_tensor("ro", [DT, 128, VOCAB], WDT, kind="ExternalInput")
    cosT_d = nc.dram_tensor("cosT", [DT, 128, T], WDT, kind="ExternalInput")
    sinT_d = nc.dram_tensor("sinT", [DT, 128, T], WDT, kind="ExternalInput")
    ident_d = nc.dram_tensor("ident", [128, 128], MDT, kind="ExternalInput")
    logits_d = nc.dram_tensor("logits", [TT, 128, VOCAB], FP,
                              kind="ExternalOutput")

    groups = [[2 * i, 2 * i + 1] for i in range(n_cores // 2)]

    with tile.TileContext(nc) as tc:
        with (
            tc.tile_pool(name="pers", bufs=1) as pers,
            tc.tile_pool(name="wk", bufs=3) as wk,
            tc.tile_pool(name="lat", bufs=2) as latp,
            tc.tile_pool(name="sm", bufs=4) as sm,
            tc.tile_pool(name="col", bufs=6) as col,
            tc.tile_pool(name="ps", bufs=4, space="PSUM") as ps,
            tc.tile_pool(name="acc", bufs=2, space="PSUM") as acc,
            tc.tile_pool(name="dram", bufs=1, space="DRAM") as dram,
        ):
            env = dict(nc=nc, cfg=cfg, MDT=MDT, WDT=WDT, wk=wk, sm=sm, col=col,
                       ps=ps, acc=acc, latp=latp, groups=groups, eh_d=eh_d,
                       cosT_d=cosT_d, sinT_d=sinT_d)

            # ---------- persistent SBUF ----------
            ident = pers.tile([128, 128], MDT, tag="ident", name="ident")
            nc.sync.dma_start(ident[:], ident_d[:])
            env["ident"] = ident

            eps_col = pers.tile([128, 1], FP, tag="eps", name="eps_col")
            nc.vector.memset(eps_col[:], EPS)
            env["eps_col"] = eps_col

            idxf = pers.tile([1, T], FP, tag="idxf", name="idxf")
            nc.sync.dma_start(idxf[:], idxf_d[:])
            wte = []
            for i in range(VT):
                w = pers.tile([128, D], FP, tag=f"wte{i}", name=f"wte{i}")
                nc.sync.dma_start(w[:], wte_d[i])
                wte.append(w)

            env["dxh"] = dxh = []
            env["dyh"] = dyh = []
            for i in range(DT):
                dx = pers.tile([128, NH], WDT, tag=f"dxh{i}", name=f"dxh{i}")
                dy = pers.tile([128, NH], WDT, tag=f"dyh{i}", name=f"dyh{i}")
                nc.sync.dma_start(dx[:], dxh_d[i])
                nc.sync.dma_start(dy[:], dyh_d[i])
                dxh.append(dx)
                dyh.append(dy)

            ro = []
            for i in range(DT):
                r = pers.tile([128, VOCAB], WDT, tag=f"ro{i}", name=f"ro{i}")
                nc.sync.dma_start(r[:], ro_d[i])
                ro.append(r)

            if cfg.get("w_dt") == "bf16":
                env["ehs"] = ehs = []
                for m in range(NH // 128):
                    e = pers.tile([128, D], WDT, tag=f"ehs{m}", name=f"ehs{m}")
                    nc.sync.dma_start(e[:], eh_d[m])
                    ehs.append(e)
            else:
                env["ehs"] = None

            env["v_sb"] = v_sb = [
                pers.tile([128, D], MDT, tag=f"v{t}", name=f"v{t}")
                for t in range(TT)]
            env["vT"] = vT = [
                pers.tile([128, T], WDT, tag=f"vT{i}", name=f"vT{i}")
                for i in range(DT)]
            env["qT"] = [
                pers.tile([128, T], MDT, tag=f"qT{i}", name=f"qT{i}")
                for i in range(DT)]

            # ---------- embedding ----------
            lnwte = []
            for i in range(VT):
                lw = pers.tile([128, D], MDT, tag=f"lnwte{i}", name=f"lnwte{i}")
                _ln_rows(env, lw, wte[i], D)
                lnwte.append(lw)

            ones1 = pers.tile([1, 128], FP, tag="ones1", name="ones1")
            nc.vector.memset(ones1[:], 1.0)
            iotav = []
            for i in range(VT):
                iv = pers.tile([128, 1], FP, tag=f"iotav{i}", name=f"iotav{i}")
                nc.gpsimd.iota(iv[:], pattern=[[0, 1]], base=i * 128,
                               channel_multiplier=1,
                               allow_small_or_imprecise_dtypes=True)
                iotav.append(iv)

            # onehotT[v, t] = (idx[t] == v), built and consumed per 512-chunk
            with tc.tile_pool(name="emb", bufs=2) as embp:
                for c in range(T // 512):
                    cs = slice(c * 512, (c + 1) * 512)
                    pidx = ps.tile([128, 512], FP, tag="mm", name="pidx")
                    nc.tensor.matmul(pidx[:], ones1[:], idxf[:, cs],
                                     start=True, stop=True)
                    oh = []
                    for i in range(VT):
                        ohi = embp.tile([128, 512], MDT, tag="ohs", name="ohs")
                        nc.vector.tensor_scalar(ohi[:], pidx[:], iotav[i][:],
                                                None, op0=ALU.is_equal)
                        oh.append(ohi)
                    # v0 = LN(wte)[idx] for the 4 token tiles of this chunk
                    for tl in range(4):
                        t = c * 4 + tl
                        pv = ps.tile([128, D], FP, tag="mm", name="pv")
                        for i in range(VT):
                            nc.tensor.matmul(pv[:],
                                             (oh[i][:, tl * 128:(tl + 1) * 128]),
                                             (lnwte[i][:]),
                                             start=(i == 0), stop=(i == VT - 1))
                        nc.vector.tensor_copy(v_sb[t][:], pv[:])
                    for i in range(DT):
                        pvt = ps.tile([128, 512], FP, tag="mm", name="pvt")
                        for k in range(VT):
                            nc.tensor.matmul(
                                pvt[:],
                                (lnwte[k][:, i * 128:(i + 1) * 128]),
                                (oh[k][:]),
                                start=(k == 0), stop=(k == VT - 1))
                        nc.vector.tensor_copy(vT[i][:, cs], pvt[:])

            env["up_dram"] = dram.tile([T, D], FP, name="upd")
            env["upr_dram"] = dram.tile([T, D], FP, name="uprd")

            # ---------- layers ----------
            total_layers = cfg["reps"] * L
            for li in range(total_layers):
                env["_layers_left"] = total_layers - 1 - li
                _emit_layer(env)

            # ---------- readout ----------
            for t in range(TT):
                pl = ps.tile([128, VOCAB], FP, tag="mm", name="pl")
                for i in range(DT):
                    nc.tensor.matmul(pl[:],
                                     (vT[i][:, t * 128:(t + 1) * 128]),
                                     (ro[i][:]),
                                     start=(i == 0), stop=(i == DT - 1))
                lg = wk.tile([128, VOCAB], FP, tag="lg", name="lg")
                nc.vector.tensor_copy(lg[:], pl[:])
                nc.sync.dma_start(logits_d[t], lg[:])

    nc.compile()
    return nc


def _ln_rows(env, out_ap, in_ap, F, resid_ap=None):
    """LN over the free dim per partition row. If resid_ap: out = resid + ln(in)."""
    nc, sm, col = env["nc"], env["sm"], env["col"]
    st6 = col.tile([128, 6], FP, tag="bst", name="bst")
    nc.vector.bn_stats(st6[:], in_ap[:])
    st2 = col.tile([128, 2], FP, tag="bag", name="bag")
    nc.vector.bn_aggr(st2[:], st6[:])
    std = col.tile([128, 1], FP, tag="std", name="std")
    nc.scalar.activation(std[:], st2[:, 1:2], ACTF.Sqrt, bias=env["eps_col"][:])
    rstd = col.tile([128, 1], FP, tag="rstd", name="rstd")
    nc.vector.reciprocal(rstd[:], std[:])
    if resid_ap is None:
        nc.vector.tensor_scalar(out_ap[:], in_ap[:], st2[:, 0:1], rstd[:],
                                op0=ALU.subtract, op1=ALU.mult)
    else:
        tmp = sm.tile([128, F], FP, tag="lntmp", name="lntmp")
        nc.vector.tensor_scalar(tmp[:], in_ap[:], st2[:, 0:1], rstd[:],
                                op0=ALU.subtract, op1=ALU.mult)
        nc.vector.tensor_add(out_ap[:], resid_ap[:], tmp[:])


def _emit_rope(env, g):
    nc, WDT = env["nc"], env["WDT"]
    cfg = env["cfg"]
    TC = cfg["TCHUNK"]
    DT = cfg["D"] // 128
    wk, vT, qT = env["wk"], env["vT"], env["qT"]
    cosT_d, sinT_d = env["cosT_d"], env["sinT_d"]
    cs = slice(g * TC, (g + 1) * TC)
    for i in range(DT):
        o = 1 - i
        ctab = wk.tile([128, TC], WDT, tag="ctab", bufs=2, name="ctab")
        nc.sync.dma_start(ctab[:], cosT_d[i, :, cs])
        stab = wk.tile([128, TC], WDT, tag="stab", bufs=2, name="stab")
        nc.sync.dma_start(stab[:], sinT_d[i, :, cs])
        t1 = wk.tile([128, TC], FP, tag="rope", bufs=2, name="ropeA")
        nc.vector.tensor_mul(t1[:], vT[i][:, cs], ctab[:])
        t2 = wk.tile([128, TC], FP, tag="rope", bufs=2, name="ropeB")
        nc.vector.tensor_mul(t2[:], vT[o][:, cs], stab[:])
        if i == 0:
            nc.vector.tensor_sub(qT[i][:, cs], t1[:], t2[:])
        else:
            nc.vector.tensor_add(qT[i][:, cs], t1[:], t2[:])


def _emit_layer(env):
    nc, cfg, MDT = env["nc"], env["cfg"], env["MDT"]
    WDT = env["WDT"]
    T, D = cfg["T"], cfg["D"]
    NH = cfg["N"] // 2
    TC = cfg["TCHUNK"]
    G = T // TC
    TT = T // 128
    MC = NH // 128
    NSUB = TC // 128
    DT = D // 128
    wk, sm, col, ps, acc = env["wk"], env["sm"], env["col"], env["ps"], env["acc"]
    vT, qT, v_sb, latp = env["vT"], env["qT"], env["v_sb"], env["latp"]
    dxh, dyh = env["dxh"], env["dyh"]
    ident, eh_d = env["ident"], env["eh_d"]
    cosT_d, sinT_d = env["cosT_d"], env["sinT_d"]
    up_dram, upr_dram = env["up_dram"], env["upr_dram"]
    XPRE = 3                              # x m-chunks emitted ahead of y

    def emit_x(m, t0, xrs):
        x_ps = ps.tile([128, TC], FP, tag="mm", name="x_ps")
        for i in range(DT):
            nc.tensor.matmul(x_ps[:],
                             dxh[i][:, m * 128:(m + 1) * 128],
                             vT[i][:, t0:t0 + TC],
                             start=(i == 0), stop=(i == DT - 1))
        xr = wk.tile([128, TC], FP, tag="xr", bufs=4, name="xr")
        nc.scalar.activation(xr[:], x_ps[:], ACTF.Relu)
        xrs[m] = xr

    def emit_av(sb, e_sb, aT_ps, nkb):
        # aT[d, t] += v[s, d]^T e[s, t]  (free dim TC keeps f32r at full rate)
        for i in range(DT):
            nc.tensor.matmul(aT_ps[:, i, :],
                             v_sb[sb][:, i * 128:(i + 1) * 128],
                             e_sb[:],
                             start=(sb == 0), stop=(sb == nkb - 1),
                             skip_group_check=True)

    for g in range(G):
        t0 = g * TC
        nkb = (g + 1) * NSUB              # causal key-block count
        # ---- RoPE for this chunk (chunk 0 may have been emitted by prev layer)
        if not (g == 0 and env.pop("_rope0_done", False)):
            _emit_rope(env, g)
        # ---- attention: energyT[s, t] blocks, AV lags two blocks (PE pipelining)
        aT_ps = acc.tile([128, DT, TC], FP, tag="acc", name="aT_ps")
        pend = []
        for sb in range(nkb):
            e_ps = ps.tile([128, TC], FP, tag="mm", name="e_ps")
            for i in range(DT):
                nc.tensor.matmul(e_ps[:],
                                 qT[i][:, sb * 128:(sb + 1) * 128],
                                 qT[i][:, t0:t0 + TC],
                                 start=(i == 0), stop=(i == DT - 1))
            e_sb = wk.tile([128, TC], MDT, tag="esb", bufs=3, name="e_sb")
            nc.vector.tensor_copy(e_sb[:], e_ps[:])
            diag_j = sb - (nkb - NSUB)
            if diag_j >= 0:
                # causal mask: keep where t - s - 128*j >= 0
                nc.gpsimd.affine_select(e_sb[:], e_sb[:], pattern=[[1, TC]],
                                        compare_op=ALU.is_ge, fill=0.0,
                                        base=-128 * diag_j,
                                        channel_multiplier=-1)
            pend.append((sb, e_sb))
            if len(pend) > 2:
                s0, e0 = pend.pop(0)
                emit_av(s0, e0, aT_ps, nkb)
        xrs = {}
        emit_x(0, t0, xrs)
        for s0, e0 in pend:
            emit_av(s0, e0, aT_ps, nkb)
        pend = []
        emit_x(1, t0, xrs)
        # ---- aT -> a (PE transposes), LN(a), la -> laT slices ----
        aT_sb = [wk.tile([128, TC], MDT, tag=f"aTs{i}", bufs=2, name="aT_sb")
                 for i in range(DT)]
        for i in range(DT):
            nc.scalar.copy(aT_sb[i][:], aT_ps[:, i, :])
        emit_x(2, t0, xrs)
        laT = [latp.tile([128, TC], WDT, tag=f"laTs{i}", name=f"laT{i}")
               for i in range(DT)]
        for tsub in range(NSUB):
            a_ti = ps.tile([128, D], MDT, tag="mm", name="a_ti")
            for i in range(DT):
                nc.tensor.matmul(a_ti[:, i * 128:(i + 1) * 128],
                                 aT_sb[i][:, tsub * 128:(tsub + 1) * 128],
                                 ident[:], is_transpose=True,
                                 start=(i == 0), stop=(i == DT - 1),
                                 skip_group_check=True)
            la = sm.tile([128, D], MDT, tag="la", name="la")
            _ln_rows(env, la, a_ti, D)
            for i in range(DT):
                ptr = ps.tile([128, 128], MDT, tag="mm", name="ptr")
                nc.tensor.matmul(ptr[:], la[:, i * 128:(i + 1) * 128], ident[:],
                                 is_transpose=True, start=True, stop=True)
                nc.vector.tensor_copy(
                    laT[i][:, tsub * 128:(tsub + 1) * 128], ptr[:])
        # ---- MLP: y = relu(Dy^T laT) * x, updateT += Eh^T yel, x pipelined ahead
        upT_ps = acc.tile([128, DT, TC], FP, tag="acc", name="upT_ps")

        def emit_up(m, yel, ehm):
            for i in range(DT):
                nc.tensor.matmul(upT_ps[:, i, :],
                                 ehm[:, i * 128:(i + 1) * 128],
                                 yel[:],
                                 start=(m == 0), stop=(m == MC - 1),
                                 skip_group_check=True)

        pup = None
        for m in range(MC):
            if env["ehs"] is not None:
                ehm = env["ehs"][m]
            else:
                ehm = wk.tile([128, D], WDT, tag="ehst", bufs=4, name="ehm")
                nc.sync.dma_start(ehm[:], eh_d[m])
            if m + XPRE < MC:
                emit_x(m + XPRE, t0, xrs)
            y_ps = ps.tile([128, TC], FP, tag="mm", name="y_ps")
            for i in range(DT):
                nc.tensor.matmul(y_ps[:],
                                 dyh[i][:, m * 128:(m + 1) * 128],
                                 laT[i][:],
                                 start=(i == 0), stop=(i == DT - 1))
            yr = wk.tile([128, TC], FP, tag="yr", bufs=2, name="yr")
            nc.scalar.activation(yr[:], y_ps[:], ACTF.Relu)
            yel = wk.tile([128, TC], WDT, tag="yel", bufs=2, name="yel")
            nc.vector.tensor_mul(yel[:], yr[:], xrs.pop(m)[:])
            if pup is not None:
                emit_up(*pup)
            pup = (m, yel, ehm)
        emit_up(*pup)
        # ---- upT -> up (PE transposes), evacuate chunk to DRAM ----
        upT_sb = [wk.tile([128, TC], MDT, tag=f"uTs{i}", bufs=1, name="upT_sb")
                  for i in range(DT)]
        for i in range(DT):
            nc.scalar.copy(upT_sb[i][:], upT_ps[:, i, :])
        for tsub in range(NSUB):
            u_ti = ps.tile([128, D], MDT, tag="mm", name="u_ti")
            for i in range(DT):
                nc.tensor.matmul(u_ti[:, i * 128:(i + 1) * 128],
                                 upT_sb[i][:, tsub * 128:(tsub + 1) * 128],
                                 ident[:], is_transpose=True,
                                 start=(i == 0), stop=(i == DT - 1),
                                 skip_group_check=True)
            u_sb = wk.tile([128, D], FP, tag="usb", bufs=2, name="u_sb")
            nc.vector.tensor_copy(u_sb[:], u_ti[:])
            r0 = t0 + tsub * 128
            nc.sync.dma_start(up_dram[r0:r0 + 128], u_sb[:])
        # ---- AllReduce each completed half so it overlaps later chunks
        if g % 2 == 1 or g == G - 1:
            hs = slice(env.get("_ar_start", 0), (g + 1) * TC)
            env["_ar_start"] = (g + 1) * TC if g < G - 1 else 0
            if cfg.get("no_cc"):
                nc.sync.dma_start(upr_dram[hs], up_dram[hs])
            else:
                nc.gpsimd.collective_compute(
                    "AllReduce", ALU.add, replica_groups=env["groups"],
                    ins=[up_dram[hs].opt()], outs=[upr_dram[hs].opt()])

    # ---- v += LN(update); refresh vT; next layer's chunk-0 rope between halves
    if G > 1:
        nfirst = TC // 128            # rope(0) needs only vT tiles of chunk 0
        _emit_vnew(env, 0, nfirst)
        if env.get("_layers_left", 0) > 0:
            _emit_rope(env, 0)
            env["_rope0_done"] = True
        _emit_vnew(env, nfirst, TT)
    else:
        _emit_vnew(env, 0, TT)


def _emit_vnew(env, t_lo, t_hi):
    nc, MDT = env["nc"], env["MDT"]
    D = env["cfg"]["D"]
    DT = D // 128
    sm, ps = env["sm"], env["ps"]
    v_sb, vT, ident = env["v_sb"], env["vT"], env["ident"]
    upr_dram = env["upr_dram"]
    for t in range(t_lo, t_hi):
        upr = sm.tile([128, D], FP, tag="upr", name="upr")
        nc.sync.dma_start(upr[:], upr_dram[t * 128:(t + 1) * 128])
        _ln_rows(env, v_sb[t], upr, D, resid_ap=v_sb[t])
        for i in range(DT):
            ptr = ps.tile([128, 128], MDT, tag="mm", name="ptr2")
            nc.tensor.matmul(ptr[:], v_sb[t][:, i * 128:(i + 1) * 128], ident[:],
                             is_transpose=True, start=True, stop=True)
            nc.vector.tensor_copy(vT[i][:, t * 128:(t + 1) * 128], ptr[:])


# ====================== host side ======================

_BUILD_CACHE = {}


def shard_inputs(cfg, idx, wte, encoder, decoder_x, decoder_y, readout):
    """Returns the list of per-core input dicts."""
    import ml_dtypes
    wnp = ml_dtypes.bfloat16 if cfg.get("w_dt") == "bf16" else np.float32
    T, D, VOCAB = cfg["T"], cfg["D"], cfg["VOCAB"]
    NH = cfg["N"] // 2
    DT = D // 128
    VT = VOCAB // 128
    heads_per_half = cfg["H"] // 2

    inv_freq = 1.0 / (10000.0 ** (np.arange(0, D, 2, dtype=np.float64) / D))
    tpos = np.arange(T, dtype=np.float64)
    freqs = np.outer(tpos, inv_freq)
    emb = np.concatenate([freqs, freqs], axis=-1)     # [T, D]
    cosT = np.ascontiguousarray(np.cos(emb).T.astype(wnp)).reshape(DT, 128, T)
    sinT = np.ascontiguousarray(np.sin(emb).T.astype(wnp)).reshape(DT, 128, T)
    ident = np.eye(128, dtype=np.float32)

    wte_s = np.ascontiguousarray(wte.astype(np.float32)).reshape(VT, 128, D)
    ro_s = np.ascontiguousarray(readout.astype(wnp)).reshape(DT, 128, VOCAB)

    in_maps = []
    for c in range(cfg["n_cores"]):
        b, h = c // 2, c % 2
        heads = range(h * heads_per_half, (h + 1) * heads_per_half)
        dxh = np.concatenate([decoder_x[hh] for hh in heads], axis=-1)  # [D, NH]
        dyh = np.concatenate([decoder_y[hh] for hh in heads], axis=-1)
        ehh = encoder[h * NH:(h + 1) * NH]                              # [NH, D]
        in_maps.append(dict(
            idxf=np.ascontiguousarray(idx[b].astype(np.float32)[None, :]),
            wte=wte_s,
            dxh=np.ascontiguousarray(dxh.astype(wnp)).reshape(DT, 128, NH),
            dyh=np.ascontiguousarray(dyh.astype(wnp)).reshape(DT, 128, NH),
            eh=np.ascontiguousarray(ehh.astype(wnp)).reshape(NH // 128, 128, D),
            ro=ro_s,
            cosT=cosT, sinT=sinT, ident=ident,
        ))
    return in_maps


def get_program(cfg):
    key = (cfg["T"], cfg["N"], cfg["L"], cfg["mm_dt"], cfg.get("w_dt"),
           cfg["reps"], cfg["n_cores"], cfg["TCHUNK"], cfg.get("no_cc"))
    if key not in _BUILD_CACHE:
        _BUILD_CACHE[key] = build_program(cfg)
    return _BUILD_CACHE[key]


def kernel(idx, wte, encoder, decoder_x, decoder_y, readout):
    cfg = default_cfg()
    nc = get_program(cfg)
    in_maps = shard_inputs(cfg, np.asarray(idx), np.asarray(wte),
                           np.asarray(encoder), np.asarray(decoder_x),
                           np.asarray(decoder_y), np.asarray(readout))
    res = run_bass_kernel_spmd(nc, in_maps, list(range(cfg["n_cores"])))
    B, T, VOCAB = cfg["B"], cfg["T"], cfg["VOCAB"]
    out = np.empty((B, T, VOCAB), np.float32)
    for b in range(B):
        out[b] = res.results[2 * b]["logits"].reshape(T, VOCAB)
    return out



# revision 35
# speedup vs baseline: 677.1071x; 677.1071x over previous
"""Trainium2 Bass kernel for the BDH-style sparse-attention network.

Reference computation (per batch b, all fp32):
  v = LN(wte[idx])                                   [T, D]
  repeat L times:
    x   = relu(v @ Dx)                               [T, N]   (Dx: [D, N])
    a   = causal_linear_attention(v) (RoPE, no softmax, tril mask)
    y   = relu(LN(a) @ Dy) * x                       [T, N]
    v   = v + LN(y @ E)                              [T, D]   (E: [N, D])
  logits = v @ readout                               [T, VOCAB]

Sharding over 8 NeuronCores: core c -> batch b = c//2, neuron half h = c%2.
Each core holds half the neuron dim (N/2 cols of Dx/Dy, N/2 rows of E) and
computes the full attention for its batch; the partial `y @ E` update is
summed with an AllReduce over core pairs [[0,1],[2,3],[4,5],[6,7]].

Layout strategy: everything feature-major ([d, t] tiles) so no transposes are
needed around the LayerNorms; LN statistics over the feature dim are computed
with ones-vector matmuls (partition reduction) and broadcast back across
partitions with a K=1 matmul.  The causal attention uses the linear-attention
prefix form: a running state S = sum_past Kr^T V ([D, D], accumulated per
512-token chunk) provides the off-diagonal contribution in O(T D^2); only the
block-diagonal chunk needs explicit (QK^T ⊙ mask)V.  The cross-core update
AllReduce runs per half-layer in bf16 and is consumed lazily one layer later
(per-chunk v update), hiding the collective behind compute.
"""

import numpy as np

import concourse.bass as bass
import concourse.bacc as bacc
import concourse.mybir as mybir
import concourse.tile as tile
from concourse.bass_utils import run_bass_kernel_spmd

FP = mybir.dt.float32
BF = mybir.dt.bfloat16
ALU = mybir.AluOpType
ACTF = mybir.ActivationFunctionType
EPS = 1e-5


def default_cfg():
    return dict(
        T=2048, D=256, N=8192, H=4, VOCAB=256, L=6, B=4,
        TCHUNK=512,
        mm_dt="f32r",        # dtype view for fp32-side matmuls
        mlp_dt="bf16",       # "bf16" | "f32": Dx/Dy/E weights + laT/yel path
        wire_dt="bf16",      # AllReduce wire dtype: "bf16" | "f32"
        ar_gran=4,           # AllReduces per layer (2 = per half, 4 = per chunk)
        n_cores=8,
        reps=1,              # layer-stack repeats (for wall-clock timing deltas)
    )


def build_program(cfg):
    T, D, VOCAB, L = cfg["T"], cfg["D"], cfg["VOCAB"], cfg["L"]
    NH = cfg["N"] // 2
    TC = cfg["TCHUNK"]
    G = T // TC
    TT = T // 128
    DT = D // 128
    VT = VOCAB // 128
    MC = NH // 128
    NSUB = TC // 128
    n_cores = cfg["n_cores"]
    assert D == 256 and TC == 512 and T % TC == 0

    MDT = mybir.dt.float32r if cfg["mm_dt"] == "f32r" else FP
    WDT = BF if cfg["mlp_dt"] == "bf16" else MDT
    ARDT = BF if cfg["wire_dt"] == "bf16" else MDT
    TABDT = BF                      # rope cos/sin tables

    nc = bacc.Bacc("TRN2", target_bir_lowering=False, debug=False,
                   num_devices=n_cores)

    idxf_d = nc.dram_tensor("idxf", [1, T], MDT, kind="ExternalInput")
    wte_d = nc.dram_tensor("wte", [VT, 128, D], FP, kind="ExternalInput")
    dxh_d = nc.dram_tensor("dxh", [DT, 128, NH], WDT, kind="ExternalInput")
    dyh_d = nc.dram_tensor("dyh", [DT, 128, NH], WDT, kind="ExternalInput")
    eh_d = nc.dram_tensor("eh", [MC, 128, D], WDT, kind="ExternalInput")
    ro_d = nc.dram_tensor("ro", [DT, 128, VOCAB], MDT, kind="ExternalInput")
    cosT_d = nc.dram_tensor("cosT", [DT, 128, T], TABDT, kind="ExternalInput")
    sinT_d = nc.dram_tensor("sinT", [DT, 128, T], TABDT, kind="ExternalInput")
    ident_d = nc.dram_tensor("ident", [128, 128], MDT, kind="ExternalInput")
    masks_d = nc.dram_tensor("masks", [NSUB, 128, TC], MDT,
                             kind="ExternalInput")
    logits_d = nc.dram_tensor("logits", [TT, 128, VOCAB], FP,
                              kind="ExternalOutput")

    groups = [[2 * i, 2 * i + 1] for i in range(n_cores // 2)]

    with tile.TileContext(nc) as tc:
        with (
            tc.tile_pool(name="pers", bufs=1) as pers,
            tc.tile_pool(name="wk", bufs=3) as wk,
            tc.tile_pool(name="sm", bufs=2) as sm,
            tc.tile_pool(name="ps", bufs=3, space="PSUM") as ps,
            tc.tile_pool(name="aTp", bufs=1, space="PSUM") as aTp,
            tc.tile_pool(name="uTp", bufs=1, space="PSUM") as uTp,
            tc.tile_pool(name="Sp", bufs=1, space="PSUM") as Sp,
            tc.tile_pool(name="dram", bufs=1, space="DRAM") as dram,
        ):
            env = dict(nc=nc, cfg=cfg, MDT=MDT, WDT=WDT, ARDT=ARDT,
                       wk=wk, sm=sm, ps=ps, aTp=aTp, uTp=uTp, Sp=Sp,
                       groups=groups, G=G, TC=TC, DT=DT, MC=MC, NSUB=NSUB)

            # ---------- persistent SBUF ----------
            ident = pers.tile([128, 128], MDT, tag="ident", name="ident")
            nc.sync.dma_start(ident[:], ident_d[:])
            env["ident"] = ident

            eps_col = pers.tile([128, 1], FP, tag="eps", name="eps_col")
            nc.vector.memset(eps_col[:], EPS)
            env["eps_col"] = eps_col

            env["masks"] = masks = []
            for j in range(NSUB):
                mk = pers.tile([128, TC], MDT, tag=f"mask{j}", name=f"mask{j}")
                nc.sync.dma_start(mk[:], masks_d[j])
                masks.append(mk)

            # all-ones vectors, carved out of mask_0 (memset can't write
            # f32r/bf16 on HW): column TC-1 and row 0 of mask_0 are all ones
            env["ones_col"] = masks[0][:, TC - 1:TC]
            env["ones_row"] = masks[0][0:1, 0:128]
            ones_col_b = pers.tile([128, 1], BF, tag="onescb",
                                   name="ones_col_b")
            nc.vector.tensor_copy(ones_col_b[:], masks[0][:, TC - 1:TC])
            env["ones_col_b"] = ones_col_b

            env["cosT_d"], env["sinT_d"] = cosT_d, sinT_d

            env["dxh"] = dxh = []
            env["dyh"] = dyh = []
            for i in range(DT):
                dx = pers.tile([128, NH], WDT, tag=f"dxh{i}", name=f"dxh{i}")
                dy = pers.tile([128, NH], WDT, tag=f"dyh{i}", name=f"dyh{i}")
                nc.sync.dma_start(dx[:], dxh_d[i])
                nc.sync.dma_start(dy[:], dyh_d[i])
                dxh.append(dx)
                dyh.append(dy)

            env["ehs"] = ehs = []
            for m in range(MC):
                e = pers.tile([128, D], WDT, tag=f"ehs{m}", name=f"ehs{m}")
                nc.sync.dma_start(e[:], eh_d[m])
                ehs.append(e)

            env["ro"] = ro = []
            for i in range(DT):
                r = pers.tile([128, VOCAB], MDT, tag=f"ro{i}", name=f"ro{i}")
                nc.sync.dma_start(r[:], ro_d[i])
                ro.append(r)

            env["vT"] = vT = [
                pers.tile([128, T], MDT, tag=f"vT{i}", name=f"vT{i}")
                for i in range(DT)]
            env["S_sb"] = pers.tile([128, DT, D], MDT, tag="Ssb", name="S_sb")

            # ---------- embedding: vT = LN(wte)[idx]^T ----------
            lnwte = []
            with tc.tile_pool(name="emb", bufs=2) as embp:
                idxf = embp.tile([1, T], MDT, tag="idxf", bufs=1,
                                 name="idxf")
                nc.sync.dma_start(idxf[:], idxf_d[:])
                for i in range(VT):
                    w = embp.tile([128, D], FP, tag="wtet", name=f"wtet{i}")
                    nc.sync.dma_start(w[:], wte_d[i])
                    lw = pers.tile([128, D], MDT, tag=f"lnwte{i}",
                                   name=f"lnwte{i}")
                    _ln_rows(env, lw, w, D)
                    lnwte.append(lw)
                iotav = []
                for i in range(VT):
                    iv = pers.tile([128, 1], FP, tag=f"iotav{i}",
                                   name=f"iotav{i}")
                    nc.gpsimd.iota(iv[:], pattern=[[0, 1]], base=i * 128,
                                   channel_multiplier=1,
                                   allow_small_or_imprecise_dtypes=True)
                    iotav.append(iv)
                for c in range(G):
                    cs = slice(c * TC, (c + 1) * TC)
                    pidx = ps.tile([128, TC], FP, tag="mm", name="pidx")
                    nc.tensor.matmul(pidx[:], env["ones_row"], idxf[:, cs],
                                     start=True, stop=True)
                    oh = []
                    for i in range(VT):
                        ohi = embp.tile([128, TC], MDT, tag="ohs", name="ohs")
                        nc.vector.tensor_scalar(ohi[:], pidx[:], iotav[i][:],
                                                None, op0=ALU.is_equal)
                        oh.append(ohi)
                    for i in range(DT):
                        pvt = ps.tile([128, TC], FP, tag="mm", name="pvt")
                        for k in range(VT):
                            nc.tensor.matmul(
                                pvt[:],
                                lnwte[k][:, i * 128:(i + 1) * 128],
                                oh[k][:],
                                start=(k == 0), stop=(k == VT - 1))
                        nc.vector.tensor_copy(vT[i][:, cs], pvt[:])

            env["up_dram"] = [dram.tile([128, DT * TC], ARDT, tag=f"upd{g}",
                                        name=f"upd{g}") for g in range(G)]
            env["upr_dram"] = [dram.tile([128, DT * TC], ARDT, tag=f"uprd{g}",
                                         name=f"uprd{g}") for g in range(G)]

            # ---------- layers (2-deep software pipeline) ----------
            total_layers = cfg["reps"] * L
            K = total_layers * G
            pro, att = {}, {}

            def mkpro(k):
                return _make_prologue(env, k % G, first_layer=(k // G == 0))

            def mkatt(k):
                return _make_attention(env, k % G, pro[k][0])

            pro[0] = mkpro(0)
            for _, _, c in pro[0][1] + pro[0][2]:
                c()
            if K > 1:
                pro[1] = mkpro(1)
                for _, _, c in pro[1][2]:
                    c()
            for k in range(K):
                cls = []
                if k + 1 < K:
                    cls += pro[k + 1][1]
                if k + 2 < K:
                    pro[k + 2] = mkpro(k + 2)
                    cls += pro[k + 2][2]
                cls.sort(key=lambda e: (e[0], e[1]))
                _emit_mloop(env, k % G, pro[k][0], mkatt(k), cls)
                pro.pop(k, None)

            # ---------- final v update + readout ----------
            for g in range(G):
                _emit_vnew(env, g)
                for tl in range(NSUB):
                    t = g * NSUB + tl
                    pl = ps.tile([128, VOCAB], FP, tag="mm", name="pl")
                    for i in range(DT):
                        nc.tensor.matmul(pl[:],
                                         vT[i][:, t * 128:(t + 1) * 128],
                                         ro[i][:],
                                         start=(i == 0), stop=(i == DT - 1),
                                         skip_group_check=True)
                    lg = wk.tile([128, VOCAB], FP, tag="lg", name="lg")
                    nc.vector.tensor_copy(lg[:], pl[:])
                    nc.sync.dma_start(logits_d[t], lg[:])

    nc.compile()
    return nc


def _ln_rows(env, out_ap, in_ap, F):
    """Token-major LN (over the free dim) — used only for the wte embedding."""
    nc, sm = env["nc"], env["sm"]
    st6 = sm.tile([128, 6], FP, tag="bst", name="bst")
    nc.vector.bn_stats(st6[:], in_ap[:])
    st2 = sm.tile([128, 2], FP, tag="bag", name="bag")
    nc.vector.bn_aggr(st2[:], st6[:])
    std = sm.tile([128, 1], FP, tag="std", name="std")
    nc.scalar.activation(std[:], st2[:, 1:2], ACTF.Sqrt,
                         bias=env["eps_col"][:])
    rstd = sm.tile([128, 1], FP, tag="rstd", name="rstd")
    nc.vector.reciprocal(rstd[:], std[:])
    nc.vector.tensor_scalar(out_ap[:], in_ap[:], st2[:, 0:1], rstd[:],
                            op0=ALU.subtract, op1=ALU.mult)


def _ln_rows_a(env, mu_ps, sq_ps, inv_n):
    """Stage A of the feature-LN row math: DVE mean/var + ACT sqrt."""
    nc, sm = env["nc"], env["sm"]
    TC = env["TC"]
    sc = sm.tile([97, TC], FP, tag="lnsc", bufs=3, name="lnsc")
    mu, musq, var, std = sc[0:1, :], sc[32:33, :], sc[64:65, :], sc[96:97, :]
    nc.vector.tensor_scalar(mu, mu_ps, inv_n, None, op0=ALU.mult)
    nc.vector.tensor_mul(musq, mu, mu)
    nc.vector.scalar_tensor_tensor(var, sq_ps, inv_n, musq,
                                   op0=ALU.mult, op1=ALU.subtract)
    nc.scalar.activation(std, var, ACTF.Sqrt, bias=env["eps_col"][0:1, :])
    return sc


def _ln_rows_b(env, sc):
    """Stage B (staggered a few m-slots later): DVE reciprocal + m2 row."""
    nc, sm = env["nc"], env["sm"]
    TC = env["TC"]
    mu, std = sc[0:1, :], sc[96:97, :]
    rstd_r = sm.tile([1, TC], env["MDT"], tag="rstdrow", bufs=2,
                     name="rstdrow")
    with nc.allow_low_precision(reason="f32r rows tile has f32 storage"):
        nc.vector.reciprocal(rstd_r[:], std)
    m2_r = sm.tile([1, TC], env["MDT"], tag="m2row", bufs=2, name="m2row")
    nc.vector.tensor_mul(m2_r[:], mu, rstd_r[:])
    return rstd_r, m2_r


def _bcast_row(env, row):
    """Broadcast a [1, TC] SBUF row across 128 partitions via a K=1 matmul,
    then evacuate to SBUF (frees the PSUM slot fast; lets DVE consumers hit
    the all-SBUF 2x perf mode)."""
    nc, ps = env["nc"], env["ps"]
    TC = env["TC"]
    b = ps.tile([128, TC], FP, tag="mm", name="bcast")
    nc.tensor.matmul(b[:], env["ones_row"], row[:],
                     start=True, stop=True, skip_group_check=True)
    bs = env["wk"].tile([128, TC], FP, tag="bcs", bufs=4, name="bcs")
    nc.scalar.copy(bs[:], b[:])
    return bs


def _emit_vnew(env, g):
    """Serial v update for the final tail: vT[:, chunk g] += LN(update)."""
    nc, cfg = env["nc"], env["cfg"]
    wk, ps = env["wk"], env["ps"]
    TC, DT = env["TC"], env["DT"]
    D = cfg["D"]
    vT = env["vT"]
    cs = slice(g * TC, (g + 1) * TC)
    ones_w = (env["ones_col_b"][:] if env["ARDT"] == BF
              else env["ones_col"])

    upr = wk.tile([128, DT * TC], env["ARDT"], tag="upr", bufs=3, name="upr")
    nc.sync.dma_start(upr[:], env["upr_dram"][g][:])
    mu_ps = ps.tile([1, TC], FP, tag="mm", name="muv")
    for i in range(DT):
        nc.tensor.matmul(mu_ps[:], ones_w,
                         upr[:, i * TC:(i + 1) * TC],
                         start=(i == 0), stop=(i == DT - 1),
                         skip_group_check=True)
    sqs = []
    for i in range(DT):
        sq = wk.tile([128, TC], env["MDT"], tag="sq", bufs=2, name="sqv")
        nc.scalar.activation(sq[:], upr[:, i * TC:(i + 1) * TC], ACTF.Square)
        sqs.append(sq)
    sq_ps = ps.tile([1, TC], FP, tag="mm", name="sqv2")
    for i in range(DT):
        nc.tensor.matmul(sq_ps[:], env["ones_col"], sqs[i][:],
                         start=(i == 0), stop=(i == DT - 1),
                         skip_group_check=True)
    sc = _ln_rows_a(env, mu_ps[:], sq_ps[:], 1.0 / D)
    rstd_r, m2_r = _ln_rows_b(env, sc)
    rstd_b = _bcast_row(env, rstd_r)
    t1s = []
    for i in range(DT):
        t1 = wk.tile([128, TC], FP, tag="vnt", bufs=4, name="vnt")
        nc.vector.tensor_mul(t1[:], upr[:, i * TC:(i + 1) * TC], rstd_b[:])
        t1s.append(t1)
    m2_b = _bcast_row(env, m2_r)
    for i in range(DT):
        t2 = wk.tile([128, TC], FP, tag="vnt", bufs=4, name="vnt2")
        nc.vector.tensor_sub(t2[:], t1s[i][:], m2_b[:])
        nc.vector.tensor_add(vT[i][:, cs], vT[i][:, cs], t2[:])


def _make_prologue(env, g, first_layer):
    """Closures that materialize chunk g's inputs: v update (unless first
    layer), rope -> qT, bf16 vT cast, v_sb/q_sb transposes, and the chunk's
    S-state contribution.  Scheduled two m-loops ahead of consumption."""
    nc, cfg = env["nc"], env["cfg"]
    MDT, WDT = env["MDT"], env["WDT"]
    wk, ps = env["wk"], env["ps"]
    TC, DT, NSUB = env["TC"], env["DT"], env["NSUB"]
    D = cfg["D"]
    vT, ident = env["vT"], env["ident"]
    cs = slice(g * TC, (g + 1) * TC)
    t0 = g * TC
    ones_w = env["ones_col_b"] if env["ARDT"] == BF else env["ones_col"]

    st = dict(g=g, qT=[], vTb=[], v_sb=[], q_sb=[], S_ps=None)
    cl = []        # list of (m_slot, tiebreak, closure)
    cl_early = []  # drained one m-loop earlier (before the next AR is
                   # emitted — Tile orders DRAM reads after every collective
                   # emitted so far, so a late read picks up a false wait)
    box = {}

    def c_upr():
        upr = wk.tile([128, DT * TC], env["ARDT"], tag="upr", bufs=3,
                      name="upr")
        nc.gpsimd.dma_start(upr[:], env["upr_dram"][g][:])
        box["upr"] = upr
    if not first_layer:
        cl_early.append((20, 2, c_upr))

    def c_dma():
        tabs = []
        for i in range(DT):
            ct = wk.tile([128, TC], BF, tag="ctab", bufs=2, name="ctab")
            nc.sync.dma_start(ct[:], env["cosT_d"][i, :, cs])
            stt = wk.tile([128, TC], BF, tag="stab", bufs=2, name="stab")
            nc.sync.dma_start(stt[:], env["sinT_d"][i, :, cs])
            tabs.append((ct, stt))
        box["tabs"] = tabs
    cl.append((1, 1, c_dma))

    if not first_layer:
        def c_sq():
            upr = box["upr"]
            sqs = []
            for i in range(DT):
                sq = wk.tile([128, TC], MDT, tag="sq", bufs=2, name="sqv")
                nc.scalar.activation(sq[:], upr[:, i * TC:(i + 1) * TC],
                                     ACTF.Square)
                sqs.append(sq)
            box["sqs"] = sqs
        cl.append((5, 1, c_sq))

        def c_stats():
            upr = box["upr"]
            mu_ps = ps.tile([1, TC], FP, tag="mm", name="muv")
            for i in range(DT):
                nc.tensor.matmul(mu_ps[:], ones_w,
                                 upr[:, i * TC:(i + 1) * TC],
                                 start=(i == 0), stop=(i == DT - 1),
                                 skip_group_check=True)
            sq_ps = ps.tile([1, TC], FP, tag="mm", name="sqv2")
            for i in range(DT):
                nc.tensor.matmul(sq_ps[:], env["ones_col"],
                                 box["sqs"][i][:],
                                 start=(i == 0), stop=(i == DT - 1),
                                 skip_group_check=True)
            box["rows2"] = (mu_ps, sq_ps)
        cl.append((9, 1, c_stats))

        def c_rows_a():
            mu_ps, sq_ps = box["rows2"]
            box["lnsc"] = _ln_rows_a(env, mu_ps[:], sq_ps[:], 1.0 / D)
        cl.append((11, 1, c_rows_a))

        def c_rows_b():
            box["lnr"] = _ln_rows_b(env, box["lnsc"])
        cl.append((14, 1, c_rows_b))

        def c_apply1():
            rstd_b = _bcast_row(env, box["lnr"][0])
            t1s = []
            for i in range(DT):
                t1 = wk.tile([128, TC], FP, tag="vnt", bufs=4, name="vnt")
                nc.vector.tensor_mul(t1[:], box["upr"][:, i * TC:(i + 1) * TC],
                                     rstd_b[:])
                t1s.append(t1)
            box["t1s"] = t1s
        cl.append((17, 1, c_apply1))

        def c_apply2():
            m2_b = _bcast_row(env, box["lnr"][1])
            for i in range(DT):
                t2 = wk.tile([128, TC], FP, tag="vnt", bufs=4, name="vnt2")
                nc.vector.tensor_sub(t2[:], box["t1s"][i][:], m2_b[:])
                nc.vector.tensor_add(vT[i][:, cs], vT[i][:, cs], t2[:])
        cl.append((19, 1, c_apply2))

    def mk_rope(i):
        def c_rope():
            o = 1 - i
            ct, stt = box["tabs"][i]
            t1 = wk.tile([128, TC], MDT, tag="rope", bufs=2, name="ropeA")
            nc.vector.tensor_mul(t1[:], vT[i][:, cs], ct[:])
            t2 = wk.tile([128, TC], MDT, tag="rope", bufs=2, name="ropeB")
            nc.vector.tensor_mul(t2[:], vT[o][:, cs], stt[:])
            q = wk.tile([128, TC], MDT, tag=f"qT{i}", bufs=2, name=f"qT{i}")
            if i == 0:
                nc.vector.tensor_sub(q[:], t1[:], t2[:])
            else:
                nc.vector.tensor_add(q[:], t1[:], t2[:])
            st["qT"].append(q)
        return c_rope
    cl.append((23, 1, mk_rope(0)))
    cl.append((24, 1, mk_rope(1)))

    def c_vtb():
        if WDT != MDT:
            for i in range(DT):
                vb = wk.tile([128, TC], WDT, tag=f"vTb{i}", bufs=2,
                             name=f"vTb{i}")
                nc.vector.tensor_copy(vb[:], vT[i][:, cs])
                st["vTb"].append(vb[:])
        else:
            st["vTb"] = [vT[i][:, cs] for i in range(DT)]
    cl.append((25, 1, c_vtb))

    def mk_tp(tl):
        def c_tp():
            tsl = slice(t0 + tl * 128, t0 + (tl + 1) * 128)
            pvq = ps.tile([128, 2 * D], MDT, tag="mm", name="ptvq")
            for i in range(DT):
                nc.tensor.matmul(pvq[:, i * 128:(i + 1) * 128], vT[i][:, tsl],
                                 ident[:], is_transpose=True,
                                 start=True, stop=True, skip_group_check=True)
            for i in range(DT):
                nc.tensor.matmul(pvq[:, D + i * 128:D + (i + 1) * 128],
                                 st["qT"][i][:, tl * 128:(tl + 1) * 128],
                                 ident[:], is_transpose=True,
                                 start=True, stop=True, skip_group_check=True)
            vs = wk.tile([128, D], MDT, tag="vsb", bufs=2 * NSUB, name="vsb")
            nc.scalar.copy(vs[:], pvq[:, 0:D])
            st["v_sb"].append(vs)
            qs = wk.tile([128, D], MDT, tag="qsb", bufs=2 * NSUB, name="qsb")
            nc.scalar.copy(qs[:], pvq[:, D:2 * D])
            st["q_sb"].append(qs)
        return c_tp
    for tl in range(NSUB):
        cl.append((27 + tl, 1, mk_tp(tl)))

    def c_supd():
        S_ps = env["Sp"].tile([128, DT, D], FP, tag="S", name="S_ps")
        for ik in range(DT):
            for j in range(NSUB):
                nc.tensor.matmul(S_ps[:, ik, :],
                                 st["q_sb"][j][:, ik * 128:(ik + 1) * 128],
                                 st["v_sb"][j][:],
                                 start=(j == 0), stop=(j == NSUB - 1),
                                 skip_group_check=True)
        st["S_ps"] = S_ps
    cl.append((31, 1, c_supd))

    return st, cl, cl_early


def _make_attention(env, g, pst):
    """Closures for attention + LN(a) of chunk g, consuming prologue state
    pst.  Scheduled one m-loop ahead of the chunk's own MLP."""
    nc, cfg = env["nc"], env["cfg"]
    MDT, WDT = env["MDT"], env["WDT"]
    wk, ps = env["wk"], env["ps"]
    TC, DT, NSUB = env["TC"], env["DT"], env["NSUB"]
    D = cfg["D"]
    S_sb = env["S_sb"]

    ast = dict(g=g, laT=[])
    cl = []
    box = {}
    e_sb = [None] * NSUB

    def c_aopen():
        aT_ps = env["aTp"].tile([128, DT, TC], FP, tag="aT", name="aT_ps")
        box["aT_ps"] = aT_ps
        if g > 0:
            for iv in range(DT):
                for ik in range(DT):
                    nc.tensor.matmul(aT_ps[:, iv, :],
                                     S_sb[:, ik, iv * 128:(iv + 1) * 128],
                                     pst["qT"][ik][:],
                                     start=(ik == 0), stop=False,
                                     skip_group_check=True)
    cl.append((2, 0, c_aopen))

    def mk_energy(j):
        def c_energy():
            c0 = j * 128
            qT = pst["qT"]
            e_ps = ps.tile([128, TC], FP, tag="mm", name="e_ps")
            for i in range(DT):
                nc.tensor.matmul(e_ps[:, c0:TC], qT[i][:, c0:c0 + 128],
                                 qT[i][:, c0:TC],
                                 start=(i == 0), stop=(i == DT - 1),
                                 skip_group_check=True)
            es = wk.tile([128, TC], MDT, tag="esb", bufs=3, name="esb")
            nc.vector.tensor_mul(es[:, c0:TC], e_ps[:, c0:TC],
                                 env["masks"][j][:, c0:TC])
            e_sb[j] = es
        return c_energy

    def mk_av(j):
        def c_av():
            c0 = j * 128
            for i in range(DT):
                nc.tensor.matmul(box["aT_ps"][:, i, c0:TC],
                                 pst["v_sb"][j][:, i * 128:(i + 1) * 128],
                                 e_sb[j][:, c0:TC],
                                 start=(g == 0 and j == 0),
                                 stop=(j == NSUB - 1 and i == DT - 1),
                                 skip_group_check=True)
        return c_av

    cl.append((3, 0, mk_energy(0)))
    cl.append((4, 0, mk_energy(1)))
    cl.append((6, 0, mk_av(0)))
    cl.append((7, 0, mk_energy(2)))
    cl.append((8, 0, mk_av(1)))
    cl.append((9, 0, mk_energy(3)))
    cl.append((10, 0, mk_av(2)))
    cl.append((11, 0, mk_av(3)))

    def c_sadd():
        if g == 0:
            nc.vector.tensor_copy(S_sb[:], pst["S_ps"][:])
        else:
            nc.vector.tensor_add(S_sb[:], S_sb[:], pst["S_ps"][:])
    cl.append((12, 0, c_sadd))

    def c_evac():
        aT_sb = [wk.tile([128, TC], MDT, tag=f"aTs{i}", bufs=1, name="aT_sb")
                 for i in range(DT)]
        for i in range(DT):
            nc.scalar.copy(aT_sb[i][:], box["aT_ps"][:, i, :])
        box["aT_sb"] = aT_sb
    cl.append((13, 0, c_evac))

    def c_sqa():
        sqa = []
        for i in range(DT):
            sq = wk.tile([128, TC], MDT, tag="sq", bufs=2, name="sqa")
            nc.scalar.activation(sq[:], box["aT_sb"][i][:], ACTF.Square)
            sqa.append(sq)
        box["sqa"] = sqa
    cl.append((14, 0, c_sqa))

    def c_stats():
        mu_ps = ps.tile([1, TC], FP, tag="mm", name="mua")
        for i in range(DT):
            nc.tensor.matmul(mu_ps[:], env["ones_col"],
                             box["aT_sb"][i][:],
                             start=(i == 0), stop=(i == DT - 1),
                             skip_group_check=True)
        sq_ps = ps.tile([1, TC], FP, tag="mm", name="sqa2")
        for i in range(DT):
            nc.tensor.matmul(sq_ps[:], env["ones_col"],
                             box["sqa"][i][:],
                             start=(i == 0), stop=(i == DT - 1),
                             skip_group_check=True)
        box["rows2"] = (mu_ps, sq_ps)
    cl.append((16, 0, c_stats))

    def c_rows_a():
        mu_ps, sq_ps = box["rows2"]
        box["lnsc"] = _ln_rows_a(env, mu_ps[:], sq_ps[:], 1.0 / D)
    cl.append((18, 0, c_rows_a))

    def c_rows_b():
        box["lnr"] = _ln_rows_b(env, box["lnsc"])
    cl.append((21, 0, c_rows_b))

    def c_lapply1():
        rstd_b = _bcast_row(env, box["lnr"][0])
        tts = []
        for i in range(DT):
            tt = wk.tile([128, TC], FP, tag="tmp", bufs=2, name="lat")
            nc.vector.tensor_mul(tt[:], box["aT_sb"][i][:], rstd_b[:])
            tts.append(tt)
        box["tts"] = tts
    cl.append((24, 0, c_lapply1))

    def c_lapply2():
        m2_b = _bcast_row(env, box["lnr"][1])
        for i in range(DT):
            la = wk.tile([128, TC], WDT, tag=f"laT{i}", bufs=2,
                         name=f"laT{i}")
            nc.vector.tensor_sub(la[:], box["tts"][i][:], m2_b[:])
            ast["laT"].append(la)
    cl.append((26, 0, c_lapply2))

    return ast, cl


def _emit_mloop(env, g, pst, att_cl, fillers):
    """Attention (inline, with x-prefill weaving) + MLP m-loop + update tail
    for chunk g; `fillers` are the next chunks' prologue closures,
    interleaved into the m-loop by m-slot."""
    nc, cfg = env["nc"], env["cfg"]
    WDT = env["WDT"]
    wk, ps = env["wk"], env["ps"]
    TC, DT, MC, G = env["TC"], env["DT"], env["MC"], env["G"]
    dxh, dyh, ehs = env["dxh"], env["dyh"], env["ehs"]
    vTb = pst["vTb"]

    xrs = {}

    def emit_x(m):
        x_ps = ps.tile([128, TC], FP, tag="mm", name="x_ps")
        for i in range(DT):
            nc.tensor.matmul(x_ps[:], dxh[i][:, m * 128:(m + 1) * 128],
                             vTb[i],
                             start=(i == 0), stop=(i == DT - 1),
                             skip_group_check=True)
        xr = wk.tile([128, TC], WDT, tag="xr", bufs=7, name="xr")
        nc.scalar.activation(xr[:], x_ps[:], ACTF.Relu)
        xrs[m] = xr

    # attention inline: weave x prefills into the LN chain for PE cover
    ast, acl = att_cl
    XAT = {13: (0, 1), 16: (2, 3), 18: (4,)}
    for slot, _, fn in acl:
        fn()
        for m in XAT.get(slot, ()):
            if m < MC:
                emit_x(m)
    laT = ast["laT"]

    upT_ps = env["uTp"].tile([128, DT, TC], FP, tag="uT", name="upT_ps")
    XPRE = 5
    fi = 0
    for m in range(MC):
        if m + XPRE < MC:
            emit_x(m + XPRE)
        y_ps = ps.tile([128, TC], FP, tag="mm", name="y_ps")
        for i in range(DT):
            nc.tensor.matmul(y_ps[:], dyh[i][:, m * 128:(m + 1) * 128],
                             laT[i][:],
                             start=(i == 0), stop=(i == DT - 1),
                             skip_group_check=True)
        yel = wk.tile([128, TC], WDT, tag="yel", bufs=3, name="yel")
        nc.vector.scalar_tensor_tensor(yel[:], y_ps[:], 0.0, xrs.pop(m)[:],
                                       op0=ALU.max, op1=ALU.mult)
        for i in range(DT):
            nc.tensor.matmul(upT_ps[:, i, :],
                             ehs[m][:, i * 128:(i + 1) * 128], yel[:],
                             start=(m == 0), stop=(m == MC - 1),
                             skip_group_check=True)
        while fi < len(fillers) and fillers[fi][0] <= m:
            fillers[fi][2]()
            fi += 1
    while fi < len(fillers):
        fillers[fi][2]()
        fi += 1

    # ---- tail: evacuate update, AllReduce when due ----
    upw = wk.tile([128, DT * TC], env["ARDT"], tag="upw", bufs=2, name="upw")
    nc.scalar.copy(upw[:, 0:TC], upT_ps[:, 0, :])
    nc.vector.tensor_copy(upw[:, TC:2 * TC], upT_ps[:, 1, :])
    nc.sync.dma_start(env["up_dram"][g][:], upw[:])

    if cfg.get("no_cc"):
        nc.sync.dma_start(env["upr_dram"][g][:], env["up_dram"][g][:])
    else:
        nc.gpsimd.collective_compute(
            "AllReduce", ALU.add, replica_groups=env["groups"],
            ins=[env["up_dram"][g][:].opt()],
            outs=[env["upr_dram"][g][:].opt()])


# ====================== host side ======================

_BUILD_CACHE = {}


def shard_inputs(cfg, idx, wte, encoder, decoder_x, decoder_y, readout):
    """Returns the list of per-core input dicts."""
    import ml_dtypes
    wnp = ml_dtypes.bfloat16 if cfg.get("mlp_dt") == "bf16" else np.float32
    T, D, VOCAB = cfg["T"], cfg["D"], cfg["VOCAB"]
    NH = cfg["N"] // 2
    TC = cfg["TCHUNK"]
    DT = D // 128
    VT = VOCAB // 128
    NSUB = TC // 128
    heads_per_half = cfg["H"] // 2

    inv_freq = 1.0 / (10000.0 ** (np.arange(0, D, 2, dtype=np.float64) / D))
    tpos = np.arange(T, dtype=np.float64)
    freqs = np.outer(tpos, inv_freq)
    emb = np.concatenate([freqs, freqs], axis=-1)     # [T, D]
    tabnp = ml_dtypes.bfloat16
    cosT = np.ascontiguousarray(np.cos(emb).T.astype(tabnp)).reshape(DT, 128, T)
    sinT = np.ascontiguousarray(np.sin(emb).T.astype(tabnp)).reshape(DT, 128, T)
    ident = np.eye(128, dtype=np.float32)
    s_i = np.arange(128)[:, None]
    t_i = np.arange(TC)[None, :]
    masks = np.stack([(t_i >= s_i + 128 * j).astype(np.float32)
                      for j in range(NSUB)])

    wte_s = np.ascontiguousarray(wte.astype(np.float32)).reshape(VT, 128, D)
    ro_s = np.ascontiguousarray(readout.astype(np.float32)) \
        .reshape(DT, 128, VOCAB)

    in_maps = []
    for c in range(cfg["n_cores"]):
        b, h = c // 2, c % 2
        heads = range(h * heads_per_half, (h + 1) * heads_per_half)
        dxh = np.concatenate([decoder_x[hh] for hh in heads], axis=-1)
        dyh = np.concatenate([decoder_y[hh] for hh in heads], axis=-1)
        ehh = encoder[h * NH:(h + 1) * NH]                # [NH, D]
        in_maps.append(dict(
            idxf=np.ascontiguousarray(idx[b].astype(np.float32)[None, :]),
            wte=wte_s,
            dxh=np.ascontiguousarray(dxh.astype(wnp)).reshape(DT, 128, NH),
            dyh=np.ascontiguousarray(dyh.astype(wnp)).reshape(DT, 128, NH),
            eh=np.ascontiguousarray(ehh.astype(wnp)).reshape(NH // 128, 128,
                                                            D),
            ro=ro_s,
            cosT=cosT, sinT=sinT, ident=ident, masks=masks,
        ))
    return in_maps


def get_program(cfg):
    key = (cfg["T"], cfg["N"], cfg["L"], cfg["mm_dt"], cfg.get("mlp_dt"),
           cfg.get("wire_dt"), cfg.get("ar_gran"), cfg["reps"],
           cfg["n_cores"], cfg["TCHUNK"], cfg.get("no_cc"))
    if key not in _BUILD_CACHE:
        _BUILD_CACHE[key] = build_program(cfg)
    return _BUILD_CACHE[key]


def kernel(idx, wte, encoder, decoder_x, decoder_y, readout):
    cfg = default_cfg()
    nc = get_program(cfg)
    in_maps = shard_inputs(cfg, np.asarray(idx), np.asarray(wte),
                           np.asarray(encoder), np.asarray(decoder_x),
                           np.asarray(decoder_y), np.asarray(readout))
    res = run_bass_kernel_spmd(nc, in_maps, list(range(cfg["n_cores"])))
    B, T, VOCAB = cfg["B"], cfg["T"], cfg["VOCAB"]
    out = np.empty((B, T, VOCAB), np.float32)
    for b in range(B):
        out[b] = res.results[2 * b]["logits"].reshape(T, VOCAB)
    return out
